# revision 17
# baseline (speedup 1.0000x reference)
"""Trainium2 Bass kernel for nn_Attention_43190191129190.

Model (per batch element b of 8):
    y   = x + dwconv3x3(x) + conv_b          (depthwise residual positional conv)
    qkv = y @ qkv_w.T ; split into q, k, v   (8 heads, dim 32)
    out = softmax(q k^T / sqrt(32)) v
    out = out @ out_w.T + out_b

Sharding: pure data-parallel, one batch element per NeuronCore (8 cores).

Per-core design (everything in transposed [C, N] space so the depthwise conv
is 9 diagonal matmuls and q^T/k^T come out in the layout the S^T matmul wants):

  1. x [1024,256] -> PE transpose -> x^T zero-padded to [C, 34, 34] in SBUF.
  2. conv: per 128-channel tile, 9 matmuls with diagonal weight matrices
     (stationary = diag(conv_w tap), moving = shifted window of padded x^T),
     accumulated in PSUM; +1.0 folded into center tap (residual); bias via a
     K=1 matmul with a ones row.  -> y^T [c, n] in SBUF.
  3. q^T,k^T [feature, token]: stationary = qkv_w^T chunks, moving = y^T.
     Head h lives at partition offset 32*(h%4) of feature tile h//4.
  4. v [token, feature] with a per-head ones column interleaved ([v_h|1]):
     stationary = y^T chunks, moving = qkv_w^T.
  5. Per head pair (0,2),(1,3),(4,6),(5,7):
       S^T[m,n] = k_h^T.T @ q_h^T via K=32 row-tiled matmuls (2 heads packed
       into different 32-row groups of the PE array);
       exp on ScalarE straight from PSUM (scale=1/sqrt(32) folded in, no max
       subtraction -- S is in [-11, 11] for this input distribution);
       PV: stationary = [v_h|1] (M=33), moving = exp(S^T) tiles, accumulated
       over the 8 m-chunks into psum rows 0:33 (fp32r requires partition-0
       dst); the ones column yields the softmax denominators in row 32.
       Normalization: reciprocal(sums) on DVE (PSUM->SBUF), gpsimd
       partition_broadcast, one vector multiply; heads whose attn^T rows are
       not 0:32 are repositioned with a cheap SBUF->SBUF DMA (which, unlike
       DVE, can shift partitions).
  6. out-projection: stationary = attn^T chunks, moving = out_w^T; bias via
     K=1 ones-row matmul; copy to SBUF; DMA out.

All matmuls use float32r (full-rate fp32 PE mode); accumulation is fp32 PSUM.
"""

import os

import numpy as np

import concourse.bass as bass
import concourse.tile as tile
from concourse import bacc, mybir
from concourse.bass_utils import run_bass_kernel_spmd

F32 = mybir.dt.float32
F32R = mybir.dt.float32r
AF = mybir.ActivationFunctionType

B, N, C = 8, 1024, 256
HEADS, DH = 8, 32
SCALE = DH ** -0.5
PAD = 34  # 32x32 spatial grid with 1-px halo

TAPS = [(ky, kx) for ky in range(3) for kx in range(3)]
PAIRS = [(0, 2), (1, 3), (4, 6), (5, 7)]


def build_nc(debug_dump=False):
    nc = bacc.Bacc("TRN2", target_bir_lowering=False, debug=False, num_devices=8)

    x_d = nc.dram_tensor("x", (N, C), F32, kind="ExternalInput").ap()
    qkvwT_d = nc.dram_tensor("qkv_wT", (C, 3 * C), F32R, kind="ExternalInput").ap()
    outwT_d = nc.dram_tensor("out_wT", (C, C), F32R, kind="ExternalInput").ap()
    diag_d = nc.dram_tensor("conv_diag", (2, 9, 128, 128), F32R, kind="ExternalInput").ap()
    convb_d = nc.dram_tensor("conv_b_r", (1, C), F32R, kind="ExternalInput").ap()
    outb_d = nc.dram_tensor("out_b_r", (1, C), F32R, kind="ExternalInput").ap()
    ones_d = nc.dram_tensor("ones_row", (1, N), F32R, kind="ExternalInput").ap()
    id_d = nc.dram_tensor("id128", (128, 128), F32, kind="ExternalInput").ap()
    out_d = nc.dram_tensor("out", (N, C), F32, kind="ExternalOutput").ap()
    dbg = {}
    if debug_dump:
        for name, shape in (
            ("d_yT", (128, 2, N)), ("d_qT", (128, 2, N)), ("d_kT", (128, 2, N)),
            ("d_v", (128, 8, 8 * 33)), ("d_attnT", (128, 2, N)),
            ("d_pv0", (128, 2048)), ("d_rs0", (128, 2048)), ("d_bc0", (128, 2048)),
            ("d_p00", (128, 1024)),
        ):
            dbg[name] = nc.dram_tensor(name, shape, F32, kind="ExternalOutput").ap()

    with tile.TileContext(nc) as tc:
        with (
            tc.tile_pool(name="const", bufs=1) as const,
            tc.tile_pool(name="xin", bufs=3) as xin_p,
            tc.tile_pool(name="big", bufs=1) as big,
            tc.tile_pool(name="pT", bufs=8) as ppool,
            tc.tile_pool(name="rs", bufs=2) as rs_p,
            tc.tile_pool(name="bc", bufs=2) as bc_p,
            tc.tile_pool(name="tmp", bufs=2) as tmp_p,
            tc.tile_pool(name="outs", bufs=3) as outs_p,
            tc.tile_pool(name="dscr", bufs=4, space="DRAM") as dram_p,
            tc.tile_pool(name="pst", bufs=2, space="PSUM") as pst,
            tc.tile_pool(name="ppv", bufs=1, space="PSUM") as ppv,
        ):
            # ---- constants ----
            id_sb = const.tile([128, 128], F32, tag="id")
            nc.sync.dma_start(id_sb, id_d)
            qkvwT_sb = const.tile([128, 2, 3 * C], F32R, tag="qkvwT")
            nc.sync.dma_start(qkvwT_sb, qkvwT_d.rearrange("(kc p) f -> p kc f", p=128))
            outwT_sb = const.tile([128, 2, C], F32R, tag="outwT")
            nc.sync.dma_start(outwT_sb, outwT_d.rearrange("(kc p) f -> p kc f", p=128))
            diag_sb = const.tile([128, 18, 128], F32R, tag="diag")
            nc.sync.dma_start(diag_sb, diag_d.rearrange("ct t p f -> p (ct t) f"))
            convb_sb = const.tile([1, C], F32R, tag="convb")
            nc.sync.dma_start(convb_sb, convb_d)
            outb_sb = const.tile([1, C], F32R, tag="outb")
            nc.sync.dma_start(outb_sb, outb_d)
            ones_sb = const.tile([1, N], F32R, tag="ones")
            nc.sync.dma_start(ones_sb, ones_d)
            zerob_sb = const.tile([128, 1], F32, tag="zerob")
            nc.vector.memset(zerob_sb, 0.0)

            # ---- persistent activations ----
            xpadT = big.tile([128, 2, PAD * PAD], F32R, tag="xpadT")
            # memset via uint32 view: walrus rejects Memset with f32r dtype
            nc.gpsimd.memset(xpadT.bitcast(mybir.dt.uint32), 0)
            yT = big.tile([128, 2, N], F32R, tag="yT")
            qT = big.tile([128, 2, N], F32R, tag="qT")
            kT = big.tile([128, 2, N], F32R, tag="kT")
            vsb = big.tile([128, 8, 8 * 33], F32R, tag="v")
            # 1.0 everywhere (ones columns); v cols overwritten below
            nc.gpsimd.memset(vsb.bitcast(mybir.dt.uint32), 0x3F800000)
            attnT = big.tile([128, 2, N], F32R, tag="attnT")

            # ---- load x, transpose into padded x^T ----
            for nt in range(8):
                xin = xin_p.tile([128, C], F32, tag="xin")
                nc.sync.dma_start(xin, x_d[nt * 128:(nt + 1) * 128, :])
                tp = pst.tile([128, 1024], F32, tag="ps")
                for ct in range(2):
                    nc.tensor.transpose(
                        tp[:, 512 * ct: 512 * ct + 128],
                        xin[:, 128 * ct: 128 * (ct + 1)],
                        id_sb,
                    )
                    dst = xpadT[:, ct, :].rearrange("p (h w) -> p h w", h=PAD)[
                        :, 1 + 4 * nt: 5 + 4 * nt, 1:33
                    ]
                    nc.vector.tensor_copy(
                        dst,
                        tp[:, 512 * ct: 512 * ct + 128].rearrange(
                            "p (a b) -> p a b", a=4
                        ),
                    )

            # ---- depthwise conv (+residual +bias) -> y^T ----
            for ct in range(2):
                cps = pst.tile([128, 1024], F32, tag="ps")
                view = xpadT[:, ct, :].rearrange("p (h w) -> p h w", h=PAD)
                for j in range(2):  # halves of the 1024 spatial positions
                    for t, (ky, kx) in enumerate(TAPS):
                        nc.tensor.matmul(
                            cps[:, j * 512:(j + 1) * 512],
                            lhsT=diag_sb[:, ct * 9 + t, :],
                            rhs=view[:, ky + 16 * j: ky + 16 * j + 16, kx: kx + 32],
                            start=(t == 0),
                            stop=False,
                        )
                    nc.tensor.matmul(
                        cps[:, j * 512:(j + 1) * 512],
                        lhsT=convb_sb[0:1, 128 * ct: 128 * (ct + 1)],
                        rhs=ones_sb[0:1, j * 512:(j + 1) * 512],
                        start=False,
                        stop=True,
                    )
                nc.vector.tensor_copy(yT[:, ct, :], cps)

            # ---- q^T, k^T in [feature, token] layout ----
            # order: heads 0-3 (q then k) first so pair work can start early
            for ft in (0, 2, 1, 3):
                dstT, dc = (qT, ft) if ft < 2 else (kT, ft - 2)
                fofs = 0 if ft < 2 else 256
                qps = pst.tile([128, 1024], F32, tag="ps")
                for j in range(2):
                    for kc in range(2):
                        nc.tensor.matmul(
                            qps[:, j * 512:(j + 1) * 512],
                            lhsT=qkvwT_sb[:, kc, fofs + dc * 128: fofs + (dc + 1) * 128],
                            rhs=yT[:, kc, j * 512:(j + 1) * 512],
                            start=(kc == 0),
                            stop=(kc == 1),
                        )
                nc.vector.tensor_copy(dstT[:, dc, :], qps)

            # ---- v in [token, feature] with interleaved ones columns ----
            for nt in range(8):
                vps = pst.tile([128, 1024], F32, tag="ps")
                for kc in range(2):
                    nc.tensor.matmul(
                        vps[:, 0:256],
                        lhsT=yT[:, kc, nt * 128:(nt + 1) * 128],
                        rhs=qkvwT_sb[:, kc, 512:768],
                        start=(kc == 0),
                        stop=(kc == 1),
                    )
                vv = vsb[:, nt, :].rearrange("p (hh c) -> p hh c", c=33)
                sv = vps[:, 0:256].rearrange("p (hh c) -> p hh c", c=32)
                nc.vector.tensor_copy(vv[:, :, 0:32], sv)  # [v_h | 1] per head

            # ---- attention, head pair at a time ----
            for ip, (hA, hB) in enumerate(PAIRS):
                pv = ppv.tile([128, 2048], F32, tag="pv")
                for m in range(8):
                    stA = pst.tile([128, 1024], F32, tag="ps")
                    stB = pst.tile([128, 1024], F32, tag="ps")
                    # S^T matmuls: 2 heads packed in different 32-row groups
                    for j in range(2):
                        for h, st in ((hA, stA), (hB, stB)):
                            a = 32 * (h % 4)
                            hc = h // 4
                            nc.tensor.matmul(
                                st[:, j * 512:(j + 1) * 512],
                                lhsT=kT[a:a + 32, hc, m * 128:(m + 1) * 128],
                                rhs=qT[a:a + 32, hc, j * 512:(j + 1) * 512],
                                start=True,
                                stop=True,
                                tile_position=(a, 0),
                            )
                    pA = ppool.tile([128, 1024], F32R, tag="pT")
                    pB = ppool.tile([128, 1024], F32R, tag="pT")
                    nc.scalar.activation(pA, stA, AF.Exp, bias=zerob_sb, scale=SCALE)
                    nc.scalar.activation(pB, stB, AF.Exp, bias=zerob_sb, scale=SCALE)
                    if debug_dump and ip == 0 and m == 0:
                        nc.sync.dma_start(dbg["d_p00"], pA.bitcast(F32))
                    # PV: [v_h|1] stationary (M=33), exp(S^T) moving; fp32r dst
                    # must start at partition 0, so both heads land in rows
                    # 0:33 -- head A in psum banks 0-1, head B in banks 2-3.
                    for j in range(2):
                        for h, pT, cofs in ((hA, pA, 0), (hB, pB, 1024)):
                            nc.tensor.matmul(
                                pv[0:33, cofs + j * 512: cofs + j * 512 + 512],
                                lhsT=vsb[:, m, 33 * h: 33 * h + 33],
                                rhs=pT[:, j * 512:(j + 1) * 512],
                                start=(m == 0),
                                stop=(m == 7),
                            )
                # softmax normalization for the pair (sums in row 32)
                rs = rs_p.tile([128, 2048], F32, tag="rs")
                bc = bc_p.tile([128, 2048], F32, tag="bc")
                tmpT = tmp_p.tile([128, 2048], F32R, tag="tmp")
                if debug_dump and ip == 0:
                    pvd = tmp_p.tile([128, 2048], F32, tag="pvd", name="pvd")
                    nc.vector.tensor_copy(pvd, pv)
                    nc.sync.dma_start(dbg["d_pv0"], pvd)
                for h, cofs in ((hA, 0), (hB, 1024)):
                    nc.vector.reciprocal(
                        rs[32:33, cofs:cofs + 1024], pv[32:33, cofs:cofs + 1024]
                    )
                    # broadcast the reciprocal row to 32 partitions: SBUF
                    # step-0 partition APs are illegal and gpsimd
                    # partition_broadcast misreads on HW, so round-trip
                    # through a DRAM scratch row and DMA-broadcast back.
                    rsd = dram_p.tile([1, 1024], F32, tag="rsd", name="rsd")
                    nc.sync.dma_start(rsd, rs[32:33, cofs:cofs + 1024])
                    nc.gpsimd.dma_start(
                        out=bc[0:32, cofs:cofs + 1024],
                        in_=bass.AP(
                            tensor=rsd.tensor,
                            offset=rsd.offset,
                            ap=[[0, 32]] + list(rsd.ap[1:]),
                        ),
                    )
                    row = 32 * (h % 4)
                    ic = h // 4
                    if row == 0:
                        nc.vector.tensor_mul(
                            attnT[0:32, ic, :],
                            pv[0:32, cofs:cofs + 1024],
                            bc[0:32, cofs:cofs + 1024],
                        )
                    else:
                        nc.vector.tensor_mul(
                            tmpT[0:32, cofs:cofs + 1024],
                            pv[0:32, cofs:cofs + 1024],
                            bc[0:32, cofs:cofs + 1024],
                        )
                        nc.sync.dma_start(
                            attnT[row:row + 32, ic, :], tmpT[0:32, cofs:cofs + 1024]
                        )

                if debug_dump and ip == 0:
                    nc.sync.dma_start(dbg["d_rs0"], rs)
                    nc.sync.dma_start(dbg["d_bc0"], bc)

            if debug_dump:
                nc.sync.dma_start(dbg["d_yT"], yT.bitcast(F32))
                nc.sync.dma_start(dbg["d_qT"], qT.bitcast(F32))
                nc.sync.dma_start(dbg["d_kT"], kT.bitcast(F32))
                nc.sync.dma_start(dbg["d_v"], vsb.bitcast(F32))
                nc.sync.dma_start(dbg["d_attnT"], attnT.bitcast(F32))

            # ---- out projection + bias ----
            for nt in range(8):
                ops = pst.tile([128, 1024], F32, tag="ps")
                for ic2 in range(2):
                    nc.tensor.matmul(
                        ops[:, 0:256],
                        lhsT=attnT[:, ic2, nt * 128:(nt + 1) * 128],
                        rhs=outwT_sb[:, ic2, :],
                        start=(ic2 == 0),
                        stop=False,
                    )
                nc.tensor.matmul(
                    ops[:, 0:256],
                    lhsT=ones_sb[0:1, 0:128],
                    rhs=outb_sb,
                    start=False,
                    stop=True,
                )
                osb = outs_p.tile([128, C], F32, tag="o")
                nc.vector.tensor_copy(osb, ops[:, 0:256])
                nc.sync.dma_start(out_d[nt * 128:(nt + 1) * 128, :], osb)

    nc.compile()
    return nc


_NC = None
LAST_RESULTS = None


def _host_prep(conv_w, conv_b, qkv_w, out_w, out_b):
    conv_w = np.asarray(conv_w, np.float32).reshape(C, 3, 3)
    diag = np.zeros((2, 9, 128, 128), np.float32)
    idx = np.arange(128)
    for ct in range(2):
        for t, (ky, kx) in enumerate(TAPS):
            d = conv_w[128 * ct: 128 * (ct + 1), ky, kx].copy()
            if (ky, kx) == (1, 1):
                d += 1.0  # residual connection folded into the center tap
            diag[ct, t, idx, idx] = d
    return {
        "qkv_wT": np.ascontiguousarray(np.asarray(qkv_w, np.float32).T),
        "out_wT": np.ascontiguousarray(np.asarray(out_w, np.float32).T),
        "conv_diag": diag,
        "conv_b_r": np.asarray(conv_b, np.float32).reshape(1, C),
        "out_b_r": np.asarray(out_b, np.float32).reshape(1, C),
        "ones_row": np.ones((1, N), np.float32),
        "id128": np.eye(128, dtype=np.float32),
    }


def kernel(x, conv_w, conv_b, qkv_w, out_w, out_b):
    global _NC, LAST_RESULTS
    if _NC is None:
        _NC = build_nc()
    x = np.asarray(x, np.float32)
    shared = _host_prep(conv_w, conv_b, qkv_w, out_w, out_b)
    in_maps = [{**shared, "x": np.ascontiguousarray(x[b])} for b in range(B)]
    trace = bool(int(os.environ.get("KERNEL_TRACE", "0")))
    res = run_bass_kernel_spmd(_NC, in_maps, core_ids=list(range(B)), trace=trace)
    LAST_RESULTS = res
    return np.stack([res.results[b]["out"] for b in range(B)], axis=0)


# revision 30
# speedup vs baseline: 444.8005x; 444.8005x over previous
"""Trainium2 Bass kernel for nn_Attention_43190191129190.

Model (per batch element b of 8):
    y   = x + dwconv3x3(x) + conv_b          (depthwise residual positional conv)
    qkv = y @ qkv_w.T ; split into q, k, v   (8 heads, dim 32)
    out = softmax(q k^T / sqrt(32)) v
    out = out @ out_w.T + out_b

Sharding: pure data-parallel, one batch element per NeuronCore (8 cores).

Per-core design (everything in transposed [C, N] space so the depthwise conv
is 9 diagonal matmuls and q^T/k^T come out in the layout the S^T matmul wants):

  1. x [1024,256] -> PE transpose -> x^T zero-padded to [C, 34, 34] in SBUF.
  2. conv: per 128-channel tile, 9 matmuls with diagonal weight matrices
     (stationary = diag(conv_w tap), moving = shifted window of padded x^T),
     accumulated in PSUM; +1.0 folded into center tap (residual); bias via a
     K=1 matmul with a ones row.  -> y^T [c, n] in SBUF.
  3. q^T,k^T [feature, token]: stationary = qkv_w^T chunks, moving = y^T.
     Head h lives at partition offset 32*(h%4) of feature tile h//4.
  4. v [token, feature] with a per-head ones column interleaved ([v_h|1]):
     stationary = y^T chunks, moving = qkv_w^T.
  5. Per head pair (two heads with different h%4 so their S^T matmuls pack
     into different 32-row groups of the PE array):
       S^T[m,n] = k_h^T.T @ q_h^T via K=32 row-tiled matmuls;
       exp on ScalarE straight from PSUM (scale=1/sqrt(32) folded in, no max
       subtraction -- S is in [-11, 11] for this input distribution);
       PV: stationary = [v_h|1] (M=33), moving = exp(S^T) tiles, accumulated
       over the 8 m-chunks into psum rows 0:33 (fp32r requires a partition-0
       dst); the ones column yields the softmax denominators in row 32.
       The PV matmuls lag the exp stream so a blocked PV (pair boundary)
       never stalls ScalarE, and each pair's psum is evacuated by a single
       DVE copy so the slot frees immediately.
       Normalization: reciprocal(sums), broadcast to 32 partitions (DMA
       round-trip through a DRAM scratch row -- SBUF APs cannot have step-0
       partitions and gpsimd partition_broadcast misreads on HW), one vector
       multiply; heads whose attn^T rows are not 0:32 are repositioned with
       a SBUF->SBUF DMA (which, unlike DVE, can shift partitions).  The last
       pair instead broadcasts on the now-idle PE and evacuates via ScalarE
       to shorten the tail.
  6. out-projection: stationary = attn^T chunks, moving = out_w^T; the
     chunk-0 half runs mid-kernel into an SBUF staging tile, chunk-1 + bias
     (K=1 ones-row matmul) + the staged half finish the tail.

All matmuls use float32r (full-rate fp32 PE mode); accumulation is fp32 PSUM.
Remaining work (v projection, q/k feature tiles 1 and 3, chunk-0 projection)
is interleaved one slice per m-step into the pair loops so the in-order PE
queue fills DMA-gated gaps instead of delaying the first exp.
"""

import os

import numpy as np

import concourse.bass as bass
import concourse.tile as tile
from concourse import bacc, mybir
from concourse.bass_utils import run_bass_kernel_spmd

F32 = mybir.dt.float32
F32R = mybir.dt.float32r
AF = mybir.ActivationFunctionType

B, N, C = 8, 1024, 256
HEADS, DH = 8, 32
SCALE = DH ** -0.5
PAD = 34  # 32x32 spatial grid with 1-px halo

TAPS = [(ky, kx) for ky in range(3) for kx in range(3)]
# order: first two pairs complete attn^T chunk 0 (heads 0-3); last pair has a
# row-0 head (4) so only one tail DMA-repositioning remains
PAIRS = [(1, 3), (0, 2), (5, 7), (4, 6)]


def build_nc(debug_dump=False):
    nc = bacc.Bacc("TRN2", target_bir_lowering=False, debug=False, num_devices=8)

    x_d = nc.dram_tensor("x", (N, C), F32, kind="ExternalInput").ap()
    qkvwT_d = nc.dram_tensor("qkv_wT", (C, 3 * C), F32R, kind="ExternalInput").ap()
    outwT_d = nc.dram_tensor("out_wT", (C, C), F32R, kind="ExternalInput").ap()
    diag_d = nc.dram_tensor("conv_diag", (2, 9, 128, 128), F32R, kind="ExternalInput").ap()
    convb_d = nc.dram_tensor("conv_b_r", (1, C), F32R, kind="ExternalInput").ap()
    outb_d = nc.dram_tensor("out_b_r", (1, C), F32R, kind="ExternalInput").ap()
    ones_d = nc.dram_tensor("ones_row", (1, N), F32R, kind="ExternalInput").ap()
    id_d = nc.dram_tensor("id128", (128, 128), F32, kind="ExternalInput").ap()
    out_d = nc.dram_tensor("out", (N, C), F32, kind="ExternalOutput").ap()
    dbg = {}
    if debug_dump:
        for name, shape in (
            ("d_yT", (128, 2, N)), ("d_qT", (128, 2, N)), ("d_kT", (128, 2, N)),
            ("d_v", (128, 8, 8 * 33)), ("d_attnT", (128, 2, N)),
        ):
            dbg[name] = nc.dram_tensor(name, shape, F32, kind="ExternalOutput").ap()

    with tile.TileContext(nc) as tc:
        with (
            tc.tile_pool(name="const", bufs=1) as const,
            tc.tile_pool(name="xin", bufs=1) as xin_p,
            tc.tile_pool(name="big", bufs=1) as big,
            tc.tile_pool(name="pT", bufs=8) as ppool,
            tc.tile_pool(name="rs", bufs=2) as rs_p,
            tc.tile_pool(name="bc", bufs=2) as bc_p,
            tc.tile_pool(name="tmp", bufs=2) as tmp_p,
            tc.tile_pool(name="outs", bufs=3) as outs_p,
            tc.tile_pool(name="dscr", bufs=4, space="DRAM") as dram_p,
            tc.tile_pool(name="pst", bufs=2, space="PSUM") as pst,
            tc.tile_pool(name="ppv", bufs=1, space="PSUM") as ppv,
        ):
            # ---- DMAs: id128 + x tiles first (startup critical path),
            # weights after; x loads spread over three DGE queues
            id_sb = const.tile([128, 128], F32, tag="id")
            nc.sync.dma_start(id_sb, id_d)
            xins = []
            _dma_engines = [nc.sync, nc.scalar, nc.sync, nc.gpsimd]
            for nt in range(8):
                xin = xin_p.tile([128, C], F32, tag=f"xin{nt}", name=f"xin{nt}")
                _dma_engines[nt % 4].dma_start(xin, x_d[nt * 128:(nt + 1) * 128, :])
                xins.append(xin)
            diag_sb = const.tile([128, 18, 128], F32R, tag="diag")
            nc.sync.dma_start(diag_sb, diag_d.rearrange("ct t p f -> p (ct t) f"))
            convb_sb = const.tile([1, C], F32R, tag="convb")
            nc.sync.dma_start(convb_sb, convb_d)
            ones_sb = const.tile([1, N], F32R, tag="ones")
            nc.sync.dma_start(ones_sb, ones_d)
            qkvwT_sb = const.tile([128, 2, 3 * C], F32R, tag="qkvwT")
            nc.sync.dma_start(qkvwT_sb, qkvwT_d.rearrange("(kc p) f -> p kc f", p=128))
            outwT_sb = const.tile([128, 2, C], F32R, tag="outwT")
            nc.sync.dma_start(outwT_sb, outwT_d.rearrange("(kc p) f -> p kc f", p=128))
            outb_sb = const.tile([1, C], F32R, tag="outb")
            nc.sync.dma_start(outb_sb, outb_d)
            zerob_sb = const.tile([128, 1], F32, tag="zerob")
            nc.vector.memset(zerob_sb, 0.0)
            # all-ones strip on every partition (PE broadcast stationary must
            # share its base partition with the moving operand)
            onesp_sb = const.tile([128, 32], F32R, tag="onesp")
            nc.gpsimd.memset(onesp_sb.bitcast(mybir.dt.uint32), 0x3F800000)

            # ---- persistent activations ----
            xpadT = big.tile([128, 2, PAD * PAD], F32R, tag="xpadT")
            # zero only the 1-px halo ring (interior is fully overwritten);
            # via a uint32 view: walrus rejects Memset with f32r dtype
            xpv = xpadT.bitcast(mybir.dt.uint32).rearrange(
                "p ct (h w) -> p ct h w", h=PAD
            )
            nc.gpsimd.memset(xpv[:, :, 0, :], 0)
            nc.gpsimd.memset(xpv[:, :, PAD - 1, :], 0)
            nc.gpsimd.memset(xpv[:, :, :, 0], 0)
            nc.gpsimd.memset(xpv[:, :, :, PAD - 1], 0)
            yT = big.tile([128, 2, N], F32R, tag="yT")
            qT = big.tile([128, 2, N], F32R, tag="qT")
            kT = big.tile([128, 2, N], F32R, tag="kT")
            vsb = big.tile([128, 8, 8 * 33], F32R, tag="v")
            # 1.0 everywhere (ones columns); v cols overwritten below
            nc.gpsimd.memset(vsb.bitcast(mybir.dt.uint32), 0x3F800000)
            attnT = big.tile([128, 2, N], F32R, tag="attnT")
            partial0 = big.tile([128, 8, C], F32, tag="partial0")

            # pre-attention psum evacuations alternate between DVE and
            # the (still idle) ScalarE so neither queue gates slot turnover
            _cp = [0]

            def copy_alt(dst, src_ap):
                _cp[0] += 1
                if _cp[0] % 2:
                    nc.vector.tensor_copy(dst, src_ap)
                else:
                    nc.scalar.copy(dst, src_ap)

            # ---- transpose x into padded x^T, conv interleaved ----
            def emit_transpose(nt):
                tp = pst.tile([128, 1024], F32, tag="ps", name="tp")
                for ct in range(2):
                    nc.tensor.transpose(
                        tp[:, 512 * ct: 512 * ct + 128],
                        xins[nt][:, 128 * ct: 128 * (ct + 1)],
                        id_sb,
                    )
                    dst = xpadT[:, ct, :].rearrange("p (h w) -> p h w", h=PAD)[
                        :, 1 + 4 * nt: 5 + 4 * nt, 1:33
                    ]
                    copy_alt(
                        dst,
                        tp[:, 512 * ct: 512 * ct + 128].rearrange(
                            "p (a b) -> p a b", a=4
                        ),
                    )

            # conv accumulators live in the (otherwise still idle) PV psum
            # slot so the transposes keep both pst slots
            cacc = ppv.tile([128, 2048], F32, tag="pv", name="cacc")

            def emit_conv_half(ct, j):
                cps = cacc[:, ct * 1024:(ct + 1) * 1024]
                view = xpadT[:, ct, :].rearrange("p (h w) -> p h w", h=PAD)
                for t, (ky, kx) in enumerate(TAPS):
                    nc.tensor.matmul(
                        cps[:, j * 512:(j + 1) * 512],
                        lhsT=diag_sb[:, ct * 9 + t, :],
                        rhs=view[:, ky + 16 * j: ky + 16 * j + 16, kx: kx + 32],
                        start=(t == 0),
                        stop=False,
                    )
                nc.tensor.matmul(
                    cps[:, j * 512:(j + 1) * 512],
                    lhsT=convb_sb[0:1, 128 * ct: 128 * (ct + 1)],
                    rhs=ones_sb[0:1, j * 512:(j + 1) * 512],
                    start=False,
                    stop=True,
                )

            # conv j=0 only needs padded rows 0..18 (x tiles 0..4), so its
            # matmuls fill the PE gaps while tiles 5..7 still stream in
            for nt in range(5):
                emit_transpose(nt)
            emit_conv_half(0, 0)
            emit_conv_half(1, 0)
            for nt in range(5, 8):
                emit_transpose(nt)
            for ct in range(2):
                emit_conv_half(ct, 1)
                copy_alt(yT[:, ct, :], cacc[:, ct * 1024:(ct + 1) * 1024])

            # ---- q^T / k^T feature tiles (heads 0-3 now; 4-7 interleaved
            # into the first pair's m-loop) ----
            def emit_qk(ft):
                dstT, dc = (qT, ft) if ft < 2 else (kT, ft - 2)
                fofs = 0 if ft < 2 else 256
                qps = pst.tile([128, 1024], F32, tag="ps", name="qps")
                for j in range(2):
                    for kc in range(2):
                        nc.tensor.matmul(
                            qps[:, j * 512:(j + 1) * 512],
                            lhsT=qkvwT_sb[:, kc, fofs + dc * 128: fofs + (dc + 1) * 128],
                            rhs=yT[:, kc, j * 512:(j + 1) * 512],
                            start=(kc == 0),
                            stop=(kc == 1),
                        )
                nc.vector.tensor_copy(dstT[:, dc, :], qps)

            def emit_v(nt):
                vps = pst.tile([128, 1024], F32, tag="ps", name="vps")
                for kc in range(2):
                    nc.tensor.matmul(
                        vps[:, 0:256],
                        lhsT=yT[:, kc, nt * 128:(nt + 1) * 128],
                        rhs=qkvwT_sb[:, kc, 512:768],
                        start=(kc == 0),
                        stop=(kc == 1),
                    )
                vv = vsb[:, nt, :].rearrange("p (hh c) -> p hh c", c=33)
                sv = vps[:, 0:256].rearrange("p (hh c) -> p hh c", c=32)
                nc.vector.tensor_copy(vv[:, :, 0:32], sv)  # [v_h | 1] per head

            def emit_proj0(nt):
                opsA = pst.tile([128, 1024], F32, tag="ps", name="opsA")
                nc.tensor.matmul(
                    opsA[:, 0:256],
                    lhsT=attnT[:, 0, nt * 128:(nt + 1) * 128],
                    rhs=outwT_sb[:, 0, :],
                    start=True,
                    stop=True,
                )
                nc.vector.tensor_copy(partial0[:, nt, :], opsA[:, 0:256])

            emit_qk(0)
            emit_qk(2)
            for nt in range(8):
                emit_v(nt)
            emit_qk(1)
            emit_qk(3)

            # chunk-0 out-projection interleaved one tile per m-step into the
            # last pair's loop (chunk 0 is long since finished by then)
            def pair_extra(ip, m):
                if ip == 3:
                    emit_proj0(m)

            # ---- attention, head pair at a time ----
            for ip, (hA, hB) in enumerate(PAIRS):
                last_pair = ip == len(PAIRS) - 1
                pv = ppv.tile([128, 2048], F32, tag="pv")

                def emit_pv(m, pA, pB, pv=pv, hA=hA, hB=hB):
                    # PV: [v_h|1] stationary (M=33), exp(S^T) moving; fp32r
                    # dst must start at partition 0, so both heads land in
                    # rows 0:33 -- head A in psum banks 0-1, head B in 2-3.
                    for j in range(2):
                        for h, pT, cofs in ((hA, pA, 0), (hB, pB, 1024)):
                            nc.tensor.matmul(
                                pv[0:33, cofs + j * 512: cofs + j * 512 + 512],
                                lhsT=vsb[:, m, 33 * h: 33 * h + 33],
                                rhs=pT[:, j * 512:(j + 1) * 512],
                                start=(m == 0),
                                stop=(m == 7),
                            )

                lag = 1 if last_pair else 2
                pend = []  # (m, pA, pB) awaiting their PV matmuls
                for m in range(8):
                    stA = pst.tile([128, 1024], F32, tag="ps")
                    stB = pst.tile([128, 1024], F32, tag="ps")
                    # S^T matmuls: 2 heads packed in different 32-row groups
                    for j in range(2):
                        for h, st in ((hA, stA), (hB, stB)):
                            a = 32 * (h % 4)
                            hc = h // 4
                            nc.tensor.matmul(
                                st[:, j * 512:(j + 1) * 512],
                                lhsT=kT[a:a + 32, hc, m * 128:(m + 1) * 128],
                                rhs=qT[a:a + 32, hc, j * 512:(j + 1) * 512],
                                start=True,
                                stop=True,
                                tile_position=(a, 0),
                            )
                    pA = ppool.tile([128, 1024], F32R, tag="pT")
                    pB = ppool.tile([128, 1024], F32R, tag="pT")
                    nc.scalar.activation(pA, stA, AF.Exp, bias=zerob_sb, scale=SCALE)
                    nc.scalar.activation(pB, stB, AF.Exp, bias=zerob_sb, scale=SCALE)
                    pair_extra(ip, m)
                    pend.append((m, pA, pB))
                    if len(pend) > lag:
                        emit_pv(*pend.pop(0))
                for e in pend:
                    emit_pv(*e)

                # ---- softmax normalization ----
                rs = rs_p.tile([128, 2048], F32, tag="rs")
                bc = bc_p.tile([128, 2048], F32, tag="bc")
                if not last_pair:
                    # evacuate pv with one DVE copy (frees the psum slot for
                    # the next pair), then normalize off-slot
                    pc = tmp_p.tile([128, 2048], F32, tag="pc", name="pc")
                    nc.vector.tensor_copy(pc[0:33, :], pv[0:33, :])
                    for h, cofs in ((hA, 0), (hB, 1024)):
                        nc.vector.reciprocal(
                            rs[32:33, cofs:cofs + 1024], pc[32:33, cofs:cofs + 1024]
                        )
                        # broadcast the reciprocal row to 32 partitions via a
                        # DRAM scratch row (SBUF step-0 partition APs are
                        # illegal; partition_broadcast misreads on HW)
                        rsd = dram_p.tile([1, 1024], F32, tag="rsd", name="rsd")
                        nc.sync.dma_start(rsd, rs[32:33, cofs:cofs + 1024])
                        row = 32 * (h % 4)
                        ic = h // 4
                        nc.gpsimd.dma_start(
                            out=bc[row:row + 32, cofs:cofs + 1024],
                            in_=bass.AP(
                                tensor=rsd.tensor,
                                offset=rsd.offset,
                                ap=[[0, 32]] + list(rsd.ap[1:]),
                            ),
                        )
                        if row == 0:
                            nc.vector.tensor_mul(
                                attnT[0:32, ic, :],
                                pc[0:32, cofs:cofs + 1024],
                                bc[0:32, cofs:cofs + 1024],
                            )
                        else:
                            # reposition to the head's attn^T rows (DMA can
                            # shift partitions; DVE cannot)
                            pcs = tmp_p.tile([128, 1024], F32, tag="pcs", name="pcs")
                            nc.sync.dma_start(
                                pcs[row:row + 32, :], pc[0:32, cofs:cofs + 1024]
                            )
                            nc.vector.tensor_mul(
                                attnT[row:row + 32, ic, :],
                                pcs[row:row + 32, :],
                                bc[row:row + 32, cofs:cofs + 1024],
                            )
                else:
                    # tail-optimized: broadcast on the now-idle PE (ones32
                    # stationary x reciprocal row), evacuate via ScalarE, and
                    # multiply straight from the pv psum (single psum operand)
                    rs2 = rs_p.tile([128, 2048], F32R, tag="rs2", name="rs2")
                    for h, cofs in ((hA, 0), (hB, 1024)):
                        nc.vector.reciprocal(
                            rs[32:33, cofs:cofs + 1024], pv[32:33, cofs:cofs + 1024]
                        )
                        # fp32r-round the reciprocal row on ScalarE (walrus
                        # requires fp32r-typed producers for matmul operands)
                        nc.scalar.copy(
                            rs2[32:33, cofs:cofs + 1024], rs[32:33, cofs:cofs + 1024]
                        )
                        bcp = pst.tile([128, 1024], F32, tag="ps", name="bcp")
                        for j in range(2):
                            nc.tensor.matmul(
                                bcp[0:32, j * 512:(j + 1) * 512],
                                lhsT=onesp_sb[32:33, :],
                                rhs=rs2[32:33, cofs + j * 512: cofs + j * 512 + 512],
                                start=True,
                                stop=True,
                            )
                        nc.scalar.copy(bc[0:32, cofs:cofs + 1024], bcp[0:32, :])
                        row = 32 * (h % 4)
                        ic = h // 4
                        if row == 0:
                            nc.vector.tensor_mul(
                                attnT[0:32, ic, :],
                                pv[0:32, cofs:cofs + 1024],
                                bc[0:32, cofs:cofs + 1024],
                            )
                        else:
                            pcs = tmp_p.tile([128, 1024], F32R, tag="pcs2", name="pcs")
                            nc.vector.tensor_mul(
                                pcs[0:32, :],
                                pv[0:32, cofs:cofs + 1024],
                                bc[0:32, cofs:cofs + 1024],
                            )
                            nc.sync.dma_start(
                                attnT[row:row + 32, ic, :], pcs[0:32, :]
                            )

            if debug_dump:
                nc.sync.dma_start(dbg["d_yT"], yT.bitcast(F32))
                nc.sync.dma_start(dbg["d_qT"], qT.bitcast(F32))
                nc.sync.dma_start(dbg["d_kT"], kT.bitcast(F32))
                nc.sync.dma_start(dbg["d_v"], vsb.bitcast(F32))
                nc.sync.dma_start(dbg["d_attnT"], attnT.bitcast(F32))

            # ---- out projection: chunk-1 half + bias + staged chunk-0 ----
            for nt in range(8):
                ops = pst.tile([128, 1024], F32, tag="ps")
                nc.tensor.matmul(
                    ops[:, 0:256],
                    lhsT=attnT[:, 1, nt * 128:(nt + 1) * 128],
                    rhs=outwT_sb[:, 1, :],
                    start=True,
                    stop=False,
                )
                nc.tensor.matmul(
                    ops[:, 0:256],
                    lhsT=ones_sb[0:1, 0:128],
                    rhs=outb_sb,
                    start=False,
                    stop=True,
                )
                osb = outs_p.tile([128, C], F32, tag="o")
                nc.vector.tensor_add(osb, ops[:, 0:256], partial0[:, nt, :])
                nc.sync.dma_start(out_d[nt * 128:(nt + 1) * 128, :], osb)

    nc.compile()
    return nc


_NC = None
LAST_RESULTS = None


def _host_prep(conv_w, conv_b, qkv_w, out_w, out_b):
    conv_w = np.asarray(conv_w, np.float32).reshape(C, 3, 3)
    diag = np.zeros((2, 9, 128, 128), np.float32)
    idx = np.arange(128)
    for ct in range(2):
        for t, (ky, kx) in enumerate(TAPS):
            d = conv_w[128 * ct: 128 * (ct + 1), ky, kx].copy()
            if (ky, kx) == (1, 1):
                d += 1.0  # residual connection folded into the center tap
            diag[ct, t, idx, idx] = d
    return {
        "qkv_wT": np.ascontiguousarray(np.asarray(qkv_w, np.float32).T),
        "out_wT": np.ascontiguousarray(np.asarray(out_w, np.float32).T),
        "conv_diag": diag,
        "conv_b_r": np.asarray(conv_b, np.float32).reshape(1, C),
        "out_b_r": np.asarray(out_b, np.float32).reshape(1, C),
        "ones_row": np.ones((1, N), np.float32),
        "id128": np.eye(128, dtype=np.float32),
    }


def kernel(x, conv_w, conv_b, qkv_w, out_w, out_b):
    global _NC, LAST_RESULTS
    if _NC is None:
        _NC = build_nc()
    x = np.asarray(x, np.float32)
    shared = _host_prep(conv_w, conv_b, qkv_w, out_w, out_b)
    in_maps = [{**shared, "x": np.ascontiguousarray(x[b])} for b in range(B)]
    trace = bool(int(os.environ.get("KERNEL_TRACE", "0")))
    res = run_bass_kernel_spmd(_NC, in_maps, core_ids=list(range(B)), trace=trace)
    LAST_RESULTS = res
    return np.stack([res.results[b]["out"] for b in range(B)], axis=0)
